# revision 8
# baseline (speedup 1.0000x reference)
"""Multi-head attention (RoPE + causal softmax) Trainium2 Bass kernel.

Problem: nn_MultiHeadAttention (B=16, S=512, D=1024, H=16, Hd=64).
Sharding: data-parallel over batch — 2 batches per core on 8 NeuronCores.

Feature-major device layout ([feature, token] tiles) so the contraction
always sits on SBUF partitions.  Per-core pipeline: q/k projections with
RoPE, v projection (ones-column augmented so attn@v also yields softmax
denominators), per-head-pair causal attention with row-group-packed score
matmuls, Wo projection.

RoPE uses the tan identity: because the RoPE tables repeat with period 32,
R @ (q * tan) * cos == rotate_half(q) * sin, so the rotation matmul
accumulates directly into the projection PSUM (start=False continuation)
and RoPE costs just 2 DVE ops per group.  The causal diagonal mask is a
single [128, 2, 128] bf16 DVE multiply covering both heads of a pair.

v4 scheduling (from trace analysis of the 227us baseline):
- Input DMA is packet-rate bound (~80ns per <=2KB partition-line per
  queue), so weights are staged host-side in an [p, m, k, c] blocked
  layout whose m-slices load with full 2KB lines; x is staged [p, k, t]
  (8KB lines); cos/tan share one [128, 1024] table.  The first qk unit's
  operands (wq/wk m=0, all of x) land ~14us in instead of ~21us.
- PE warmup: dummy N=512 matmuls bridge the framework preamble to the
  first DMA arrival so HAM is at K=8/8 when real matmuls start.
- Softmax denominator reciprocal runs on the [128,8] transposed layout
  (DVE reciprocal costs ~6.5 cyc/elem per lane; a [1,512] reciprocal is
  3.3us), with the transpose dance batched per PAIR; the two denominator
  row copies split across Scalar (h0) and Vector (h1) to balance engines.
- wo output PSUM->SBUF copies on the Scalar engine.
- Global schedule gives every attention pair ~5.5us of co-emitted dense
  matmul work (qk unit / v group / wo group) so the ~5us softmax chain
  latency hides.
"""

import numpy as np
import ml_dtypes

BF16 = ml_dtypes.bfloat16

B, S, D = 16, 512, 1024
H, HD = 16, 64
NCORES = 8
BPC = B // NCORES
T = BPC * S

_CACHE = {}


def _rope_tables():
    inv_freq = 1.0 / (10000.0 ** (np.arange(0, HD, 2, dtype=np.float64) / HD))
    t = np.arange(S, dtype=np.float64)
    freqs = np.outer(t, inv_freq)
    emb = np.concatenate([freqs, freqs], -1)
    return np.cos(emb), np.sin(emb)


def _host_consts():
    cos, sin = _rope_tables()
    tan = sin / cos
    cosT = np.tile(cos.T, (2, 1))          # [128, S]
    tanT = np.tile(tan.T, (2, 1))          # [128, S]
    costan = np.ascontiguousarray(
        np.concatenate([cosT, tanT], axis=1)).astype(BF16)  # [128, 2S]
    R64 = np.zeros((64, 64), np.float32)
    R64[np.arange(32), np.arange(32) + 32] = -1.0
    R64[np.arange(32) + 32, np.arange(32)] = 1.0
    R128 = np.zeros((128, 128), np.float32)
    R128[:64, :64] = R64
    R128[64:, 64:] = R64
    RT = np.ascontiguousarray(R128.T).astype(BF16)
    mask01 = (np.arange(128)[None, :] >= np.arange(128)[:, None]).astype(BF16)
    mask2 = np.ascontiguousarray(np.concatenate([mask01, mask01], axis=1))
    return costan, RT, mask2


def _block_weight(w):
    """WqT [D, D] -> [128, 8192] with layout [p, m, k, c]:
    out[p, m*1024 + k*128 + c] = WqT[k*128+p, m*128+c]."""
    wt = np.ascontiguousarray(w.T).astype(BF16)          # WqT [D, D]
    b = wt.reshape(8, 128, 8, 128)                        # [k, p, m, c]
    return np.ascontiguousarray(
        b.transpose(1, 2, 0, 3).reshape(128, 8192))       # [p, m, k, c]


def _build_bass(dump_debug=False):
    import concourse.bacc as bacc
    import concourse.tile as tile
    import concourse.mybir as mybir

    dt = mybir.dt
    f32, bf16 = dt.float32, dt.bfloat16
    Exp = mybir.ActivationFunctionType.Exp

    nc = bacc.Bacc("TRN2", target_bir_lowering=False, debug=False, enable_asserts=False)

    xT_d = nc.dram_tensor("xTw", [128, 8 * T], bf16, kind="ExternalInput").ap()
    wq_d = nc.dram_tensor("WqB", [128, 8192], bf16, kind="ExternalInput").ap()
    wk_d = nc.dram_tensor("WkB", [128, 8192], bf16, kind="ExternalInput").ap()
    wv_d = nc.dram_tensor("WvT", [D, D], bf16, kind="ExternalInput").ap()
    wo_d = nc.dram_tensor("WoT", [D, D], bf16, kind="ExternalInput").ap()
    ct_d = nc.dram_tensor("costan", [128, 2 * S], bf16, kind="ExternalInput").ap()
    rt_d = nc.dram_tensor("RT", [128, 128], bf16, kind="ExternalInput").ap()
    mask_d = nc.dram_tensor("mask2", [128, 256], bf16, kind="ExternalInput").ap()
    out_d = nc.dram_tensor("outT", [D, T], f32, kind="ExternalOutput").ap()

    KC = D // 128

    with tile.TileContext(nc) as tc:
        with (
            tc.tile_pool(name="consts", bufs=1) as consts,
            tc.tile_pool(name="persist", bufs=1) as persist,
            tc.tile_pool(name="work", bufs=3) as work,
            tc.tile_pool(name="expp", bufs=3) as expp,
            tc.tile_pool(name="ps_a", bufs=4, space="PSUM") as ps_a,
            tc.tile_pool(name="ps_b", bufs=2, space="PSUM") as ps_b,
        ):
            # ---- PE warmup ----
            warm = consts.tile([128, 512], bf16, name="warm")
            nc.gpsimd.memset(warm, 0.0)
            wps = ps_a.tile([128, S], f32, name="wps", tag="ps_a")
            for _ in range(26):
                nc.tensor.matmul(wps, warm[:, 0:128], warm, start=True, stop=True)

            # ---- input loads: critical set first, 2KB+ lines, queue-spread --
            RT = consts.tile([128, 128], bf16, name="RT")
            nc.sync.dma_start(out=RT, in_=rt_d)
            mask2 = consts.tile([128, 2, 128], bf16, name="mask2")
            nc.sync.dma_start(out=mask2, in_=mask_d)

            wqB = consts.tile([128, 8, 8, 128], bf16, name="wqB")
            wkB = consts.tile([128, 8, 8, 128], bf16, name="wkB")
            xTw = consts.tile([128, 8, T], bf16, name="xTw")
            # wq/wk m=0 slices, 4-way partition split each
            for p0 in range(0, 128, 32):
                nc.sync.dma_start(out=wqB[p0:p0 + 32, 0, :, :],
                                  in_=wq_d[p0:p0 + 32, 0:1024])
                nc.sync.dma_start(out=wkB[p0:p0 + 32, 0, :, :],
                                  in_=wk_d[p0:p0 + 32, 0:1024])
            # all of x, 8 quarter loads (8KB lines)
            for p0 in range(0, 128, 64):
                for k0 in range(0, 8, 4):
                    nc.sync.dma_start(out=xTw[p0:p0 + 64, k0:k0 + 4, :],
                                      in_=xT_d[p0:p0 + 64, k0 * T:(k0 + 4) * T])
            costan = consts.tile([128, 2 * S], bf16, name="costan")
            for p0 in range(0, 128, 64):
                nc.sync.dma_start(out=costan[p0:p0 + 64, :], in_=ct_d[p0:p0 + 64, :])
            cosT = costan[:, 0:S]
            tanT = costan[:, S:2 * S]
            # remaining wq/wk m-slices, 2-way partition split
            for m in range(1, 8):
                for p0 in range(0, 128, 64):
                    nc.sync.dma_start(out=wqB[p0:p0 + 64, m, :, :],
                                      in_=wq_d[p0:p0 + 64, m * 1024:(m + 1) * 1024])
                    nc.sync.dma_start(out=wkB[p0:p0 + 64, m, :, :],
                                      in_=wk_d[p0:p0 + 64, m * 1024:(m + 1) * 1024])
            wv = [consts.tile([128, D], bf16, name=f"wv{k}") for k in range(KC)]
            for k in range(KC):
                for p0 in range(0, 128, 64):
                    nc.sync.dma_start(out=wv[k][p0:p0 + 64, :],
                                      in_=wv_d[k * 128 + p0:k * 128 + p0 + 64, :])
            wo = [consts.tile([128, D], bf16, name=f"wo{k}") for k in range(KC)]
            for k in range(KC):
                for p0 in range(0, 128, 64):
                    nc.sync.dma_start(out=wo[k][p0:p0 + 64, :],
                                      in_=wo_d[k * 128 + p0:k * 128 + p0 + 64, :])

            qrot = [persist.tile([128, T], bf16, name=f"qrot{m}") for m in range(KC)]
            krot = [persist.tile([128, T], bf16, name=f"krot{m}") for m in range(KC)]
            vsb = [persist.tile([128, H * 65], bf16, name=f"vsb{t_}") for t_ in range(T // 128)]
            att = [persist.tile([128, T], bf16, name=f"att{m}") for m in range(KC)]

            for t_ in range(T // 128):
                vt = vsb[t_].rearrange("p (h w) -> p h w", w=65)
                nc.gpsimd.memset(vt[:, :, 64:65], 1.0)

            # qk projection with the RoPE tan trick (A/B split).
            def emit_qk_A(nb, w_sb, rot, m):
                cols = slice(nb * S, (nb + 1) * S)
                pp = ps_a.tile([128, S], f32, name="pp", tag="ps_a")
                for k in range(KC):
                    nc.tensor.matmul(
                        pp, w_sb[:, m, k, :], xTw[:, k, cols],
                        start=(k == 0), stop=(k == KC - 1))
                pre2 = work.tile([128, S], bf16, name="pre2", tag="pre2", bufs=2)
                nc.vector.tensor_mul(pre2, pp, tanT)
                return (pp, pre2, rot, m, cols)

            def emit_qk_B(st):
                pp, pre2, rot, m, cols = st
                nc.tensor.matmul(pp, RT, pre2, start=False, stop=True,
                                 skip_group_check=True)
                nc.vector.tensor_mul(rot[m][:, cols], pp, cosT)

            def emit_qk_unit(nb, m):
                st_q = emit_qk_A(nb, wqB, qrot, m)
                st_k = emit_qk_A(nb, wkB, krot, m)
                emit_qk_B(st_q)
                emit_qk_B(st_k)

            def emit_v_group(tch, nh):
                vt = vsb[tch].rearrange("p (h w) -> p h w", w=65)
                vp = ps_a.tile([128, S], f32, name="vp", tag="ps_a")
                for k in range(KC):
                    nc.tensor.matmul(
                        vp, xTw[:, k, tch * 128:(tch + 1) * 128],
                        wv[k][:, nh * S:(nh + 1) * S],
                        start=(k == 0), stop=(k == KC - 1))
                nc.scalar.copy(
                    vt[:, nh * 8:(nh + 1) * 8, 0:64],
                    vp.rearrange("p (h w) -> p h w", w=64))

            # attention pair; softmax denominators for BOTH heads share one
            # transpose/recip/transpose-back dance (see v3 docstring note).
            def emit_attn_pair(b, j):
                mh = j
                exs = []
                for i in range(4):
                    lo = i * 128
                    sc = ps_b.tile([128, 2, S], f32, name="sc", tag="ps_b")
                    for hi, p0 in ((0, 0), (1, 64)):
                        nc.tensor.matmul(
                            sc[:, hi, 0:S - lo],
                            krot[mh][p0:p0 + 64, b * S + lo: b * S + lo + 128],
                            qrot[mh][p0:p0 + 64, b * S + lo: (b + 1) * S],
                            start=True, stop=True)
                    ex = expp.tile([128, 2, S], bf16, name="ex", tag=f"ex{i}")
                    nc.scalar.activation(ex[:, :, lo:S], sc[:, :, 0:S - lo], Exp, scale=0.125)
                    nc.vector.tensor_mul(ex[:, :, lo:lo + 128], ex[:, :, lo:lo + 128], mask2)
                    exs.append(ex)

                avs = []
                ss2 = work.tile([1, 2, S], f32, name="ss2", tag="ss2", bufs=2)
                for hi in (0, 1):
                    h = 2 * j + hi
                    av = ps_a.tile([128, S], f32, name="av", tag="ps_a")
                    for i in range(4):
                        lo = i * 128
                        nc.tensor.matmul(
                            av[0:65, lo:S],
                            vsb[b * 4 + i][:, h * 65: h * 65 + 65],
                            exs[i][:, hi, lo:S],
                            start=(i == 0), stop=(i == 3), skip_group_check=True)
                    if hi == 0:
                        nc.scalar.copy(ss2[0:1, 0, :], av[64:65, :])
                    else:
                        nc.vector.tensor_copy(ss2[0:1, 1, :], av[64:65, :])
                    avs.append(av)
                st = work.tile([128, 8], f32, name="st", tag="st", bufs=2)
                nc.gpsimd.dma_start(out=st, in_=ss2)
                rt = work.tile([128, 8], f32, name="rt", tag="rt", bufs=2)
                nc.vector.reciprocal(rt, st)
                rr = work.tile([1, 2, S], f32, name="rr", tag="rr", bufs=2)
                nc.gpsimd.dma_start(out=rr, in_=rt)
                bcols = slice(b * S, (b + 1) * S)
                for hi in (0, 1):
                    p0 = hi * 64
                    rb = work.tile([64, S], f32, name="rb", tag="rb", bufs=2)
                    nc.gpsimd.partition_broadcast(rb, rr[0:1, hi, :])
                    nc.vector.tensor_mul(att[mh][p0:p0 + 64, bcols], avs[hi][0:64, :], rb)

            def emit_wo_group(b, m):
                bcols = slice(b * S, (b + 1) * S)
                fin = ps_a.tile([128, S], f32, name="fin", tag="ps_a")
                for k in range(KC):
                    nc.tensor.matmul(
                        fin, wo[k][:, m * 128:(m + 1) * 128], att[k][:, bcols],
                        start=(k == 0), stop=(k == KC - 1))
                ob = work.tile([128, S], f32, name="ob", tag="ob", bufs=2)
                nc.scalar.copy(ob, fin)
                nc.sync.dma_start(out=out_d[m * 128:(m + 1) * 128, bcols], in_=ob)

            # ---- global schedule ----
            for m in range(KC):
                emit_qk_unit(0, m)
            for tch in range(4):
                emit_v_group(tch, 0)
            for j in range(4):  # pairs (0,0..3)
                emit_attn_pair(0, j)
                emit_v_group(j, 1)
                emit_qk_unit(1, j)
            for j in range(4, 8):  # pairs (0,4..7)
                emit_attn_pair(0, j)
                emit_v_group(j, 0)
                emit_qk_unit(1, j)
            for j in range(4):  # pairs (1,0..3)
                emit_attn_pair(1, j)
                emit_v_group(4 + j, 1)
                emit_wo_group(0, j)
            for j in range(4, 8):  # pairs (1,4..7)
                emit_attn_pair(1, j)
                emit_wo_group(0, j)
            for m in range(KC):
                emit_wo_group(1, m)

    nc.compile()
    return nc


def _get_nc():
    if "nc" not in _CACHE:
        _CACHE["nc"] = _build_bass()
    return _CACHE["nc"]


def make_in_maps(x, Wq, Wk, Wv, Wo):
    costan, RT, mask2 = _host_consts()
    shared = {
        "WqB": _block_weight(Wq),
        "WkB": _block_weight(Wk),
        "WvT": np.ascontiguousarray(Wv.T).astype(BF16),
        "WoT": np.ascontiguousarray(Wo.T).astype(BF16),
        "costan": costan,
        "RT": RT,
        "mask2": mask2,
    }
    in_maps = []
    for c in range(NCORES):
        xc = x[c * BPC:(c + 1) * BPC]
        xT = np.ascontiguousarray(xc.transpose(2, 0, 1).reshape(D, T)).astype(BF16)
        # [p, k, t] layout: row p holds the 8 k-chunks' token rows
        xTw = np.ascontiguousarray(
            xT.reshape(8, 128, T).transpose(1, 0, 2).reshape(128, 8 * T))
        in_maps.append({"xTw": xTw, **shared})
    return in_maps


def assemble(results):
    out = np.empty((B, S, D), np.float32)
    for c in range(NCORES):
        oT = np.asarray(results[c]["outT"])
        out[c * BPC:(c + 1) * BPC] = oT.reshape(D, BPC, S).transpose(1, 2, 0)
    return out


def run(x, Wq, Wk, Wv, Wo, trace=False, **run_kwargs):
    from concourse.bass_utils import run_bass_kernel_spmd
    nc = _get_nc()
    in_maps = make_in_maps(x, Wq, Wk, Wv, Wo)
    res = run_bass_kernel_spmd(
        nc, in_maps, core_ids=list(range(NCORES)), trace=trace, **run_kwargs)
    return assemble(res.results), res


def kernel(x, Wq, Wk, Wv, Wo):
    out, _ = run(np.asarray(x), np.asarray(Wq), np.asarray(Wk),
                 np.asarray(Wv), np.asarray(Wo))
    return out
